# revision 1
# baseline (speedup 1.0000x reference)
"""Trainium2 Bass kernel for CrossAttention (B=32, N=M=1024, D=1024, DQK=128).

Computes, per batch b:
    Q = x @ Wq + bq            [N, DQK]
    K = ctx @ Wk + bk          [M, DQK]
    V = ctx @ Wv + bv          [M, D]
    S = Q @ K^T                [N, M]
    W = softmax(S, axis=-1)    [N, M]
    out = W @ V + x            [N, D]
Returns (out, W) as float32, matching the reference.

Sharding: data-parallel over batch across 8 NeuronCores (4 batches/core),
weights replicated. Each core runs an identical SPMD Bass/Tile program.

Precision: Q/K projections and scores run in fp32 (softmax is sensitive to
score error); V projection and the W@V matmul run in bf16 (PSUM accumulation
stays fp32), which is 4x faster on the PE array.
"""

import numpy as np

B, N, M, D = 32, 1024, 1024, 1024
E = 128          # DQK
P = 128          # partitions
NCORES = 8
BPC = B // NCORES
KC = D // P      # contraction chunks
NC_ = N // P     # n chunks
MC = M // P      # m chunks
H = 512          # matmul moving free-dim (one PSUM bank of fp32)

_STATE = {}


def _build(nb):
    """Build the per-core Bass/Tile program for nb batches."""
    import concourse.bass as bass
    import concourse.tile as tile
    from concourse import bacc, mybir
    from concourse.masks import make_identity

    f32 = mybir.dt.float32
    bf16 = mybir.dt.bfloat16
    AX = mybir.AxisListType
    AF = mybir.ActivationFunctionType

    nc = bacc.Bacc(None, target_bir_lowering=False, debug=False)
    x_d = nc.dram_tensor("x", [nb, N, D], f32, kind="ExternalInput")
    c_d = nc.dram_tensor("ctx", [nb, M, D], f32, kind="ExternalInput")
    wq_d = nc.dram_tensor("Wq", [D, E], f32, kind="ExternalInput")
    bq_d = nc.dram_tensor("bq", [E], f32, kind="ExternalInput")
    wk_d = nc.dram_tensor("Wk", [D, E], f32, kind="ExternalInput")
    bk_d = nc.dram_tensor("bk", [E], f32, kind="ExternalInput")
    wv_d = nc.dram_tensor("Wv", [D, D], f32, kind="ExternalInput")
    bv_d = nc.dram_tensor("bv", [D], f32, kind="ExternalInput")
    out_d = nc.dram_tensor("out", [nb, N, D], f32, kind="ExternalOutput")
    wts_d = nc.dram_tensor("wts", [nb, N, M], f32, kind="ExternalOutput")

    with tile.TileContext(nc) as tc:
        with (
            tc.tile_pool(name="const", bufs=1) as constp,
            tc.tile_pool(name="stage", bufs=3) as stagep,
            tc.tile_pool(name="xres", bufs=1) as xresp,
            tc.tile_pool(name="tposed", bufs=1) as tposedp,
            tc.tile_pool(name="ctxbf", bufs=1) as ctxbfp,
            tc.tile_pool(name="vpool", bufs=1) as vpoolp,
            tc.tile_pool(name="qk", bufs=1) as qkp,
            tc.tile_pool(name="attn", bufs=2) as attnp,
            tc.tile_pool(name="outs", bufs=3) as outsp,
            tc.tile_pool(name="small", bufs=8) as smallp,
            tc.tile_pool(name="psum_mm", bufs=3, space="PSUM") as psmm,
            tc.tile_pool(name="psum_t", bufs=2, space="PSUM") as pst,
        ):
            # ---- constants (loaded once) ----
            ident_f = constp.tile([P, P], f32)
            make_identity(nc, ident_f)
            ident_b = constp.tile([P, P], bf16)
            make_identity(nc, ident_b)

            wq_sb = constp.tile([P, KC, E], f32)
            nc.sync.dma_start(
                out=wq_sb, in_=wq_d[:, :].rearrange("(k p) e -> p k e", p=P)
            )
            wk_sb = constp.tile([P, KC, E], f32)
            nc.sync.dma_start(
                out=wk_sb, in_=wk_d[:, :].rearrange("(k p) e -> p k e", p=P)
            )
            bq_sb = constp.tile([P, 1], f32)
            nc.sync.dma_start(
                out=bq_sb, in_=bq_d[:].rearrange("(p one) -> p one", one=1)
            )
            bk_sb = constp.tile([P, 1], f32)
            nc.sync.dma_start(
                out=bk_sb, in_=bk_d[:].rearrange("(p one) -> p one", one=1)
            )
            # bv broadcast to all partitions
            bv_sb = constp.tile([P, D], f32)
            bv_ap = bv_d[:]
            bv_bcast = bass.AP(
                tensor=bv_ap.tensor, offset=bv_ap.offset, ap=[[0, P]] + list(bv_ap.ap)
            )
            nc.gpsimd.dma_start(out=bv_sb, in_=bv_bcast)
            # Wv cast to bf16, laid out as [p, k, dout]
            wv_bf = constp.tile([P, KC, D], bf16)
            for k in range(KC):
                s = stagep.tile([P, D], f32, tag="stage")
                nc.sync.dma_start(out=s, in_=wv_d[k * P : (k + 1) * P, :])
                nc.scalar.copy(wv_bf[:, k, :], s)

            for b in range(nb):
                # ---- resident x[b] (transpose source + residual) ----
                x_sb = xresp.tile([P, NC_, D], f32, tag="x")
                nc.sync.dma_start(
                    out=x_sb, in_=x_d[b].rearrange("(j p) d -> p j d", p=P)
                )

                # ---- transpose ctx[b] -> ctxT (f32) and ctx_bf (bf16) ----
                ctxT = tposedp.tile([P, KC, M], f32, tag="tposed")
                ctx_bf = ctxbfp.tile([P, KC, M], bf16, tag="ctxbf")
                for j in range(MC):
                    s = stagep.tile([P, D], f32, tag="stage")
                    nc.sync.dma_start(out=s, in_=c_d[b, j * P : (j + 1) * P, :])
                    for g in range(2):
                        pt = pst.tile([P, 4, P], f32, tag="t")
                        for u in range(4):
                            i = 4 * g + u
                            nc.tensor.transpose(
                                pt[:, u, :], s[:, i * P : (i + 1) * P], ident_f
                            )
                        nc.vector.tensor_copy(
                            ctxT[:, 4 * g : 4 * g + 4, j * P : (j + 1) * P], pt
                        )
                        nc.scalar.copy(
                            ctx_bf[:, 4 * g : 4 * g + 4, j * P : (j + 1) * P], pt
                        )

                # ---- K^T = (ctx @ Wk + bk)^T  -> [e, m] (f32) ----
                k_ps = psmm.tile([P, M], f32, tag="mm")
                for h in range(2):
                    for k in range(KC):
                        nc.tensor.matmul(
                            k_ps[:, h * H : (h + 1) * H],
                            wk_sb[:, k, :],
                            ctxT[:, k, h * H : (h + 1) * H],
                            start=(k == 0),
                            stop=(k == KC - 1),
                        )
                kT = qkp.tile([P, M], f32, tag="kT")
                nc.scalar.add(kT, k_ps, bk_sb)

                # ---- V = ctx @ Wv + bv  -> [m, dout] (bf16) ----
                v_sb = vpoolp.tile([P, MC, D], bf16, tag="v")
                for j in range(MC):
                    v_ps = psmm.tile([P, D], f32, tag="mm")
                    for h in range(2):
                        for k in range(KC):
                            nc.tensor.matmul(
                                v_ps[:, h * H : (h + 1) * H],
                                ctx_bf[:, k, j * P : (j + 1) * P],
                                wv_bf[:, k, h * H : (h + 1) * H],
                                start=(k == 0),
                                stop=(k == KC - 1),
                            )
                    nc.vector.tensor_add(v_sb[:, j, :], v_ps, bv_sb)

                # ---- transpose x[b] -> xT (f32) ----
                xT = tposedp.tile([P, KC, N], f32, tag="tposed")
                for j in range(NC_):
                    for g in range(2):
                        pt = pst.tile([P, 4, P], f32, tag="t")
                        for u in range(4):
                            i = 4 * g + u
                            nc.tensor.transpose(
                                pt[:, u, :], x_sb[:, j, i * P : (i + 1) * P], ident_f
                            )
                        nc.vector.tensor_copy(
                            xT[:, 4 * g : 4 * g + 4, j * P : (j + 1) * P], pt
                        )

                # ---- Q^T = (x @ Wq + bq)^T -> [e, n] (f32) ----
                q_ps = psmm.tile([P, N], f32, tag="mm")
                for h in range(2):
                    for k in range(KC):
                        nc.tensor.matmul(
                            q_ps[:, h * H : (h + 1) * H],
                            wq_sb[:, k, :],
                            xT[:, k, h * H : (h + 1) * H],
                            start=(k == 0),
                            stop=(k == KC - 1),
                        )
                qT = qkp.tile([P, N], f32, tag="qT")
                nc.scalar.add(qT, q_ps, bq_sb)

                # ---- attention: scores -> softmax -> W @ V + x ----
                # scores are emitted one n-chunk ahead so the PE can work on
                # chunk i+1's scores while chunk i's softmax runs on DVE/ACT.
                s_ps_list = [None] * NC_

                def emit_scores(i):
                    s_ps = psmm.tile([P, M], f32, tag="mm")
                    for h in range(2):
                        nc.tensor.matmul(
                            s_ps[:, h * H : (h + 1) * H],
                            qT[:, i * P : (i + 1) * P],
                            kT[:, h * H : (h + 1) * H],
                        )
                    return s_ps

                s_ps_list[0] = emit_scores(0)
                for i in range(NC_):
                    if i + 1 < NC_:
                        s_ps_list[i + 1] = emit_scores(i + 1)
                    s_ps = s_ps_list[i]
                    s_ps_list[i] = None

                    negmax = smallp.tile([P, 1], f32, tag="negmax")
                    nc.vector.reduce_max(negmax, s_ps, axis=AX.X, negate=True)
                    p_sb = attnp.tile([P, M], f32, tag="p")
                    sumex = smallp.tile([P, 1], f32, tag="sumex")
                    nc.scalar.activation(
                        p_sb, s_ps, AF.Exp, bias=negmax, scale=1.0, accum_out=sumex
                    )
                    rsum = smallp.tile([P, 1], f32, tag="rsum")
                    nc.vector.reciprocal(rsum, sumex)
                    # normalized weights (f32) -> DRAM
                    pw = outsp.tile([P, M], f32, tag="pw")
                    nc.scalar.activation(pw, p_sb, AF.Identity, bias=0.0, scale=rsum)
                    nc.sync.dma_start(out=wts_d[b, i * P : (i + 1) * P, :], in_=pw)
                    # normalized weights (bf16) for the W @ V matmul
                    pb = attnp.tile([P, M], bf16, tag="pb")
                    nc.vector.tensor_scalar_mul(pb, p_sb, rsum)
                    pT_ps = pst.tile([P, MC, P], bf16, tag="t")
                    for j in range(MC):
                        nc.tensor.transpose(
                            pT_ps[:, j, :], pb[:, j * P : (j + 1) * P], ident_b
                        )
                    pT = attnp.tile([P, MC, P], bf16, tag="pT")
                    nc.vector.tensor_copy(pT, pT_ps)
                    av_ps = psmm.tile([P, D], f32, tag="mm")
                    for h in range(2):
                        for j in range(MC):
                            nc.tensor.matmul(
                                av_ps[:, h * H : (h + 1) * H],
                                pT[:, j, :],
                                v_sb[:, j, h * H : (h + 1) * H],
                                start=(j == 0),
                                stop=(j == MC - 1),
                            )
                    att = outsp.tile([P, D], f32, tag="att")
                    nc.vector.tensor_add(att, av_ps, x_sb[:, i, :])
                    nc.sync.dma_start(out=out_d[b, i * P : (i + 1) * P, :], in_=att)

    return nc


def _get_program(nb):
    if nb not in _STATE:
        nc = _build(nb)
        nc.finalize()
        _STATE[nb] = nc
    return _STATE[nb]


def run(inputs, trace=False):
    """Run on 8 cores; returns (out, wts, BassKernelResults)."""
    from concourse import bass_utils

    nc = _get_program(BPC)
    x = np.ascontiguousarray(np.asarray(inputs["x"], dtype=np.float32))
    ctx = np.ascontiguousarray(np.asarray(inputs["context"], dtype=np.float32))
    shared = {
        "Wq": np.ascontiguousarray(np.asarray(inputs["Wq"], dtype=np.float32)),
        "bq": np.ascontiguousarray(np.asarray(inputs["bq"], dtype=np.float32)),
        "Wk": np.ascontiguousarray(np.asarray(inputs["Wk"], dtype=np.float32)),
        "bk": np.ascontiguousarray(np.asarray(inputs["bk"], dtype=np.float32)),
        "Wv": np.ascontiguousarray(np.asarray(inputs["Wv"], dtype=np.float32)),
        "bv": np.ascontiguousarray(np.asarray(inputs["bv"], dtype=np.float32)),
    }
    in_maps = []
    for c in range(NCORES):
        m = dict(shared)
        m["x"] = x[c * BPC : (c + 1) * BPC]
        m["ctx"] = ctx[c * BPC : (c + 1) * BPC]
        in_maps.append(m)

    kw = {}
    if trace:
        _install_ntff_hook()
        kw["trace"] = True
    res = bass_utils.run_bass_kernel_spmd(nc, in_maps, list(range(NCORES)), **kw)
    out = np.concatenate([res.results[c]["out"] for c in range(NCORES)], axis=0)
    wts = np.concatenate([res.results[c]["wts"] for c in range(NCORES)], axis=0)
    return out, wts, res


def _install_ntff_hook():
    """The container's antenv stub lacks axon_hooks; provide it so
    run_bass_kernel_spmd(trace=True) can capture NTFF profiles."""
    import sys, types

    if "antenv.axon_hooks" in sys.modules:
        return
    import antenv
    from concourse import bass_utils

    bass_utils.upload_artifacts = lambda d: d  # no artifact store here
    try:
        from trn_agent_boot.trn_boot import _ntff_profile_via_ctypes

        hook = _ntff_profile_via_ctypes("/opt/axon/libaxon_pjrt.so")
    except Exception:
        hook = None
    mod = types.ModuleType("antenv.axon_hooks")
    mod.get_axon_ntff_profile_hook = lambda: hook
    mod.set_axon_ntff_profile_hook = lambda h: None
    sys.modules["antenv.axon_hooks"] = mod
    antenv.axon_hooks = mod


def kernel(**inputs):
    out, wts, _ = run(inputs, trace=False)
    return out, wts


# revision 10
# speedup vs baseline: 1.2782x; 1.2782x over previous
"""Trainium2 Bass kernel for CrossAttention (B=32, N=M=1024, D=1024, DQK=128).

Computes, per batch b:
    Q = x @ Wq + bq            [N, DQK]
    K = ctx @ Wk + bk          [M, DQK]
    V = ctx @ Wv + bv          [M, D]
    S = Q @ K^T                [N, M]
    W = softmax(S, axis=-1)    [N, M]
    out = W @ V + x            [N, D]
Returns (out, W) as float32, matching the reference.

Sharding: data-parallel over batch across 8 NeuronCores (4 batches/core),
weights replicated. Each core runs an identical SPMD Bass/Tile program.

Precision: Q/K projections and scores run in fp32 (softmax is sensitive to
score error); V projection and the W@V matmul run in bf16 (PSUM accumulation
stays fp32), which is 4x faster on the PE array.
"""

import numpy as np

B, N, M, D = 32, 1024, 1024, 1024
E = 128          # DQK
P = 128          # partitions
NCORES = 8
BPC = B // NCORES
KC = D // P      # contraction chunks
NC_ = N // P     # n chunks
MC = M // P      # m chunks
H = 512          # matmul moving free-dim (one PSUM bank of fp32)

_STATE = {}


def _build(nb):
    """Build the per-core Bass/Tile program for nb batches."""
    import concourse.bass as bass
    import concourse.tile as tile
    from concourse import bacc, mybir
    from concourse.masks import make_identity

    f32 = mybir.dt.float32
    f32r = mybir.dt.float32r
    bf16 = mybir.dt.bfloat16
    AX = mybir.AxisListType
    AF = mybir.ActivationFunctionType

    # float32r: fp32 storage, PE runs it at 1 cycle/row (vs 4 for strict fp32)
    # with slightly reduced internal precision. Used for the Q/K/score path.
    def r(ap):
        return ap.bitcast(f32r)

    nc = bacc.Bacc(None, target_bir_lowering=False, debug=False)
    x_d = nc.dram_tensor("x", [nb, N, D], f32, kind="ExternalInput")
    c_d = nc.dram_tensor("ctx", [nb, M, D], f32, kind="ExternalInput")
    wq_d = nc.dram_tensor("Wq", [D, E], f32, kind="ExternalInput")
    bq_d = nc.dram_tensor("bq", [E], f32, kind="ExternalInput")
    wk_d = nc.dram_tensor("Wk", [D, E], f32, kind="ExternalInput")
    bk_d = nc.dram_tensor("bk", [E], f32, kind="ExternalInput")
    wv_d = nc.dram_tensor("Wv", [D, D], f32, kind="ExternalInput")
    bv_d = nc.dram_tensor("bv", [D], f32, kind="ExternalInput")
    out_d = nc.dram_tensor("out", [nb, N, D], f32, kind="ExternalOutput")
    wts_d = nc.dram_tensor("wts", [nb, N, M], f32, kind="ExternalOutput")

    with tile.TileContext(nc) as tc:
        with (
            tc.tile_pool(name="const", bufs=1) as constp,
            tc.tile_pool(name="stage", bufs=4) as stagep,
            tc.tile_pool(name="xres", bufs=1) as xresp,
            tc.tile_pool(name="tposed", bufs=1) as tposedp,
            tc.tile_pool(name="ctxbf", bufs=1) as ctxbfp,
            tc.tile_pool(name="vpool", bufs=1) as vpoolp,
            tc.tile_pool(name="qk", bufs=1) as qkp,
            tc.tile_pool(name="attn", bufs=2) as attnp,
            tc.tile_pool(name="outs", bufs=3) as outsp,
            tc.tile_pool(name="small", bufs=8) as smallp,
            tc.tile_pool(name="psum_mm", bufs=3, space="PSUM") as psmm,
            tc.tile_pool(name="psum_t", bufs=2, space="PSUM") as pst,
        ):
            # ---- constants (loaded once) ----
            ident_f = constp.tile([P, P], f32)
            make_identity(nc, ident_f)
            ident_b = constp.tile([P, P], bf16)
            make_identity(nc, ident_b)

            # f32r operands must come from an op that rounds to f32r; DMA does
            # not, so weights go through a staging tile + DVE copy.
            wq_sb = constp.tile([P, KC, E], f32r)
            sq = stagep.tile([P, D], f32, tag="stage")
            nc.sync.dma_start(
                out=sq.rearrange("p (k e) -> p k e", k=KC),
                in_=wq_d[:, :].rearrange("(k p) e -> p k e", p=P),
            )
            nc.vector.tensor_copy(wq_sb, sq.rearrange("p (k e) -> p k e", k=KC))
            wk_sb = constp.tile([P, KC, E], f32r)
            sk = stagep.tile([P, D], f32, tag="stage")
            nc.sync.dma_start(
                out=sk.rearrange("p (k e) -> p k e", k=KC),
                in_=wk_d[:, :].rearrange("(k p) e -> p k e", p=P),
            )
            nc.vector.tensor_copy(wk_sb, sk.rearrange("p (k e) -> p k e", k=KC))
            bq_sb = constp.tile([P, 1], f32)
            nc.sync.dma_start(
                out=bq_sb, in_=bq_d[:].rearrange("(p one) -> p one", one=1)
            )
            bk_sb = constp.tile([P, 1], f32)
            nc.sync.dma_start(
                out=bk_sb, in_=bk_d[:].rearrange("(p one) -> p one", one=1)
            )
            # bv broadcast to all partitions
            bv_sb = constp.tile([P, D], f32)
            bv_ap = bv_d[:]
            bv_bcast = bass.AP(
                tensor=bv_ap.tensor, offset=bv_ap.offset, ap=[[0, P]] + list(bv_ap.ap)
            )
            nc.gpsimd.dma_start(out=bv_sb, in_=bv_bcast)
            # Wv cast to bf16, laid out as [p, k, dout].  Emitted lazily (after
            # batch 0's ctx loads) so the first transposes aren't starved of DMA.
            wv_bf = constp.tile([P, KC, D], bf16)

            def emit_wv_staging():
                for k in range(KC):
                    s = stagep.tile([P, D], f32, tag="stage")
                    nc.sync.dma_start(out=s, in_=wv_d[k * P : (k + 1) * P, :])
                    nc.scalar.copy(wv_bf[:, k, :], s)

            def emit_x_load(b):
                x_sb = xresp.tile([P, NC_, D], f32, tag="x")
                nc.sync.dma_start(
                    out=x_sb, in_=x_d[b].rearrange("(j p) d -> p j d", p=P)
                )
                return x_sb

            for b in range(nb):
                # resident x[b] (transpose source + residual); for b=0 this is
                # deferred until after the ctx loads that feed the first PE work
                if b > 0:
                    x_sb = emit_x_load(b)

                # ---- transpose ctx[b] -> ctxT (f32) and ctx_bf (bf16) ----
                ctxT = tposedp.tile([P, KC, M], f32r, tag="tposed")
                ctx_bf = ctxbfp.tile([P, KC, M], bf16, tag="ctxbf")
                for j in range(MC):
                    s = stagep.tile([P, D], f32, tag="stage")
                    nc.sync.dma_start(out=s, in_=c_d[b, j * P : (j + 1) * P, :])
                    for g in range(2):
                        pt = pst.tile([P, 4, P], f32, tag="t")
                        for u in range(4):
                            i = 4 * g + u
                            nc.tensor.transpose(
                                pt[:, u, :], s[:, i * P : (i + 1) * P], ident_f
                            )
                        nc.vector.tensor_copy(
                            ctxT[:, 4 * g : 4 * g + 4, j * P : (j + 1) * P], pt
                        )
                        nc.scalar.copy(
                            ctx_bf[:, 4 * g : 4 * g + 4, j * P : (j + 1) * P], pt
                        )
                if b == 0:
                    emit_wv_staging()
                    x_sb = emit_x_load(0)

                # ---- K^T = (ctx @ Wk + bk)^T  -> [e, m] (f32) ----
                k_ps = psmm.tile([P, M], f32, tag="mm")
                for h in range(2):
                    for k in range(KC):
                        nc.tensor.matmul(
                            k_ps[:, h * H : (h + 1) * H],
                            wk_sb[:, k, :],
                            ctxT[:, k, h * H : (h + 1) * H],
                            start=(k == 0),
                            stop=(k == KC - 1),
                        )
                kT = qkp.tile([P, M], f32r, tag="kT")
                nc.scalar.add(kT, k_ps, bk_sb)

                # ---- V = ctx @ Wv + bv  -> [m, dout] (bf16) ----
                v_sb = vpoolp.tile([P, MC, D], bf16, tag="v")
                for j in range(MC):
                    v_ps = psmm.tile([P, D], f32, tag="mm")
                    for h in range(2):
                        for k in range(KC):
                            nc.tensor.matmul(
                                v_ps[:, h * H : (h + 1) * H],
                                ctx_bf[:, k, j * P : (j + 1) * P],
                                wv_bf[:, k, h * H : (h + 1) * H],
                                start=(k == 0),
                                stop=(k == KC - 1),
                            )
                    nc.vector.tensor_add(v_sb[:, j, :], v_ps, bv_sb)

                # ---- transpose x[b] -> xT (f32) ----
                xT = tposedp.tile([P, KC, N], f32r, tag="tposed")
                for j in range(NC_):
                    for g in range(2):
                        pt = pst.tile([P, 4, P], f32, tag="t")
                        for u in range(4):
                            i = 4 * g + u
                            nc.tensor.transpose(
                                pt[:, u, :], x_sb[:, j, i * P : (i + 1) * P], ident_f
                            )
                        nc.vector.tensor_copy(
                            xT[:, 4 * g : 4 * g + 4, j * P : (j + 1) * P], pt
                        )

                # ---- Q^T = (x @ Wq + bq)^T -> [e, n] (f32) ----
                q_ps = psmm.tile([P, N], f32, tag="mm")
                for h in range(2):
                    for k in range(KC):
                        nc.tensor.matmul(
                            q_ps[:, h * H : (h + 1) * H],
                            wq_sb[:, k, :],
                            xT[:, k, h * H : (h + 1) * H],
                            start=(k == 0),
                            stop=(k == KC - 1),
                        )
                qT = qkp.tile([P, N], f32r, tag="qT")
                nc.scalar.add(qT, q_ps, bq_sb)

                # ---- attention: scores -> softmax -> W @ V + x ----
                # scores are emitted one n-chunk ahead so the PE can work on
                # chunk i+1's scores while chunk i's softmax runs on DVE/ACT.
                s_ps_list = [None] * NC_

                def emit_scores(i):
                    s_ps = psmm.tile([P, M], f32, tag="mm")
                    for h in range(2):
                        nc.tensor.matmul(
                            s_ps[:, h * H : (h + 1) * H],
                            qT[:, i * P : (i + 1) * P],
                            kT[:, h * H : (h + 1) * H],
                        )
                    return s_ps

                s_ps_list[0] = emit_scores(0)
                for i in range(NC_):
                    if i + 1 < NC_:
                        s_ps_list[i + 1] = emit_scores(i + 1)
                    s_ps = s_ps_list[i]
                    s_ps_list[i] = None

                    negmax = smallp.tile([P, 1], f32, tag="negmax")
                    nc.vector.reduce_max(negmax, s_ps, axis=AX.X, negate=True)
                    p_sb = attnp.tile([P, M], f32, tag="p")
                    sumex = smallp.tile([P, 1], f32, tag="sumex")
                    nc.scalar.activation(
                        p_sb, s_ps, AF.Exp, bias=negmax, scale=1.0, accum_out=sumex
                    )
                    rsum = smallp.tile([P, 1], f32, tag="rsum")
                    nc.vector.reciprocal(rsum, sumex)
                    # normalized weights (f32) -> DRAM
                    pw = outsp.tile([P, M], f32, tag="pw")
                    nc.scalar.activation(pw, p_sb, AF.Identity, bias=0.0, scale=rsum)
                    nc.sync.dma_start(out=wts_d[b, i * P : (i + 1) * P, :], in_=pw)
                    # normalized weights (bf16) for the W @ V matmul
                    pb = attnp.tile([P, M], bf16, tag="pb")
                    nc.vector.tensor_scalar_mul(pb, p_sb, rsum)
                    pT_ps = pst.tile([P, MC, P], bf16, tag="t")
                    for j in range(MC):
                        nc.tensor.transpose(
                            pT_ps[:, j, :], pb[:, j * P : (j + 1) * P], ident_b
                        )
                    pT = attnp.tile([P, MC, P], bf16, tag="pT")
                    nc.vector.tensor_copy(pT, pT_ps)
                    av_ps = psmm.tile([P, D], f32, tag="mm")
                    for h in range(2):
                        for j in range(MC):
                            nc.tensor.matmul(
                                av_ps[:, h * H : (h + 1) * H],
                                pT[:, j, :],
                                v_sb[:, j, h * H : (h + 1) * H],
                                start=(j == 0),
                                stop=(j == MC - 1),
                            )
                    att = outsp.tile([P, D], f32, tag="att")
                    nc.vector.tensor_add(att, av_ps, x_sb[:, i, :])
                    nc.sync.dma_start(out=out_d[b, i * P : (i + 1) * P, :], in_=att)

    return nc


def _get_program(nb):
    if nb not in _STATE:
        nc = _build(nb)
        nc.finalize()
        _STATE[nb] = nc
    return _STATE[nb]


def run(inputs, trace=False):
    """Run on 8 cores; returns (out, wts, BassKernelResults)."""
    from concourse import bass_utils

    nc = _get_program(BPC)
    x = np.ascontiguousarray(np.asarray(inputs["x"], dtype=np.float32))
    ctx = np.ascontiguousarray(np.asarray(inputs["context"], dtype=np.float32))
    shared = {
        "Wq": np.ascontiguousarray(np.asarray(inputs["Wq"], dtype=np.float32)),
        "bq": np.ascontiguousarray(np.asarray(inputs["bq"], dtype=np.float32)),
        "Wk": np.ascontiguousarray(np.asarray(inputs["Wk"], dtype=np.float32)),
        "bk": np.ascontiguousarray(np.asarray(inputs["bk"], dtype=np.float32)),
        "Wv": np.ascontiguousarray(np.asarray(inputs["Wv"], dtype=np.float32)),
        "bv": np.ascontiguousarray(np.asarray(inputs["bv"], dtype=np.float32)),
    }
    in_maps = []
    for c in range(NCORES):
        m = dict(shared)
        m["x"] = x[c * BPC : (c + 1) * BPC]
        m["ctx"] = ctx[c * BPC : (c + 1) * BPC]
        in_maps.append(m)

    kw = {}
    if trace:
        _install_ntff_hook()
        kw["trace"] = True
    res = bass_utils.run_bass_kernel_spmd(nc, in_maps, list(range(NCORES)), **kw)
    out = np.concatenate([res.results[c]["out"] for c in range(NCORES)], axis=0)
    wts = np.concatenate([res.results[c]["wts"] for c in range(NCORES)], axis=0)
    return out, wts, res


def _install_ntff_hook():
    """The container's antenv stub lacks axon_hooks; provide it so
    run_bass_kernel_spmd(trace=True) can capture NTFF profiles."""
    import sys, types

    if "antenv.axon_hooks" in sys.modules:
        return
    import antenv
    from concourse import bass_utils

    bass_utils.upload_artifacts = lambda d: d  # no artifact store here
    try:
        from trn_agent_boot.trn_boot import _ntff_profile_via_ctypes

        hook = _ntff_profile_via_ctypes("/opt/axon/libaxon_pjrt.so")
    except Exception:
        hook = None
    mod = types.ModuleType("antenv.axon_hooks")
    mod.get_axon_ntff_profile_hook = lambda: hook
    mod.set_axon_ntff_profile_hook = lambda h: None
    sys.modules["antenv.axon_hooks"] = mod
    antenv.axon_hooks = mod


def kernel(**inputs):
    out, wts, _ = run(inputs, trace=False)
    return out, wts
